# revision 45
# baseline (speedup 1.0000x reference)
"""AngProtoLoss (stable) distributed Bass kernel for 8 TRN2 NeuronCores.

Problem (reference):
    dvecs: (4096, 16, 512) f32
    centroids = mean(dvecs, axis=1)                  # (N, D)
    u = dvecs[:, -1, :]                              # (N, D)
    cos = clip(cos_sim(u, centroids), min=1e-6)      # (N, N)
    logits = cos * w + b
    loss = -mean(diag(log_softmax(logits)))
        = mean_i [ logsumexp_k(w*clip(cos_ik)) - w*clip(cos_ii) ]   (b cancels)

Sharding: data-parallel over speakers N. Each core gets 512 speakers (4
chunks of 128), computes local normalized centroids (bf16 tree sum ->
rsqrt-normalize), transposes them on the TensorE, all-gathers them in fp8
(one allgather per chunk, pipelined against the load/centroid phase), then
computes its 512 rows of the cos matrix in bf16 x fp8 matmuls, applies
clip+exp (with ScalarE accumulate) for the log-sum-exp, and the local
diagonal terms. Device outputs per-row exp-sums and diagonal cos; the host
unshard does rows = log(s) - w*clip(diag) and means over N (b cancels in
log-softmax exactly).

Schedule notes (engine queues are FIFO):
 - gpsimd queue holds only the bounce writes + collectives so each allgather
   triggers as soon as its bounce is written (never stuck behind big loads).
 - X loads and gathered reads ride the sync HWDGE ring in data-ready order.
 - explicit add_dep edges keep phase-C matmuls/epilogue behind all phase-B
   work on PE/DVE/ACT queues, so a late chunk's transposes are never stalled
   behind ops waiting on a gather.
 - bounce layout is [128 d-rows x 512B (t,i)-cols] so every DMA touching
   HBM moves >=512B contiguous runs (small descriptors starve during
   collectives).
"""

import os
import sys

for _p in ("/opt/trn_rl_repo",):
    if os.path.isdir(_p) and _p not in sys.path:
        sys.path.append(_p)

import numpy as np

import concourse.bass as bass
import concourse.tile as tile
from concourse import bacc, mybir
from concourse.bass_utils import run_bass_kernel_spmd
from concourse.masks import make_identity

N_CORES = 8
N, M, D = 4096, 16, 512
P = 128                     # partitions
LOCAL = N // N_CORES        # 512 speakers per core
NCHUNK = LOCAL // P         # 4 chunks of 128 speakers
NT = D // P                 # 4 d-tiles
EPS = 1e-6

F32 = mybir.dt.float32
BF16 = mybir.dt.bfloat16
FP8 = mybir.dt.float8e4
AF = mybir.ActivationFunctionType


def build_program(w_val: float):
    nc = bacc.Bacc("TRN2", target_bir_lowering=False, debug=False,
                   num_devices=N_CORES)
    dvecs = nc.dram_tensor("dvecs", [LOCAL, M, D], F32, kind="ExternalInput").ap()
    out = nc.dram_tensor("out", [2, LOCAL], F32, kind="ExternalOutput").ap()

    with tile.TileContext(nc) as tc:
        _build(nc, tc, dvecs, out, w_val)
    nc.compile()
    return nc


def _build(nc, tc, dvecs, out, w_val):
    from contextlib import ExitStack
    ctx = ExitStack()
    with ctx:
        singles = ctx.enter_context(tc.tile_pool(name="singles", bufs=1))
        xpool = ctx.enter_context(tc.tile_pool(name="xpool", bufs=2))
        tree = ctx.enter_context(tc.tile_pool(name="tree", bufs=2))
        cpool = ctx.enter_context(tc.tile_pool(name="cpool", bufs=2))
        stats = ctx.enter_context(tc.tile_pool(name="stats", bufs=4))
        gpool = ctx.enter_context(tc.tile_pool(name="gpool", bufs=1))
        epool = ctx.enter_context(tc.tile_pool(name="epool", bufs=3))
        tpsum = ctx.enter_context(tc.tile_pool(name="tpsum", bufs=2, space="PSUM"))
        mpsum = ctx.enter_context(tc.tile_pool(name="mpsum", bufs=2, space="PSUM"))
        dram = ctx.enter_context(tc.tile_pool(name="dram", bufs=1, space="DRAM"))

        ident = singles.tile([P, P], F32)
        make_identity(nc, ident)

        # persistent across the whole kernel
        uT = singles.tile([P, NT, LOCAL], BF16)          # u^T: [d_in_tile, t, i]
        s_acc = singles.tile([P, NCHUNK], F32)           # sum_k exp(w*clip(cos))
        diag_all = singles.tile([P, NCHUNK], F32)        # diag cos, per q
        nc.vector.memset(s_acc, 0.0)

        # ---------- phase A: loads first (sync ring order) ----------
        xs = []
        for r in range(NCHUNK):
            x = xpool.tile([P, M, D], F32, name=f"x{r}", tag="x")
            nc.sync.dma_start(out=x, in_=dvecs[r * P:(r + 1) * P, :, :])
            xs.append(x)

        # ---------- phase B: per-chunk centroid pipeline + allgather ----------
        # One allgather per chunk (grouping chunks into fewer, bigger AGs
        # measured slower: the 1MB fp8 AG falls into the slow RDH regime).
        GROUPS = [[0], [1], [2], [3]]
        chunk_group = {}
        for gi, grp in enumerate(GROUPS):
            for slot, rr in enumerate(grp):
                chunk_group[rr] = (gi, slot)
        bounces = [None] * len(GROUPS)
        cc_insts = []
        gath = []
        last_transpose = [None]
        last_dve_b = [None]
        last_act_b = [None]
        for r in range(NCHUNK):
            x = xs[r]
            # centroid sum over m: first level casts f32 -> bf16
            t1 = tree.tile([P, M // 2, D], BF16, name=f"t1_{r}", tag="t1")
            for j in range(M // 2):
                nc.vector.tensor_add(t1[:, j, :], x[:, 2 * j, :], x[:, 2 * j + 1, :])
            t2 = tree.tile([P, M // 4, D], BF16, name=f"t2_{r}", tag="t2")
            for j in range(M // 4):
                nc.vector.tensor_add(t2[:, j, :], t1[:, 2 * j, :], t1[:, 2 * j + 1, :])
            t3 = tree.tile([P, M // 8, D], BF16, name=f"t3_{r}", tag="t3")
            for j in range(M // 8):
                nc.vector.tensor_add(t3[:, j, :], t2[:, 2 * j, :], t2[:, 2 * j + 1, :])
            csum = cpool.tile([P, D], BF16, name=f"csum{r}", tag="csum")
            nc.vector.tensor_add(csum, t3[:, 0, :], t3[:, 1, :])

            u = x[:, M - 1, :]   # last utterance (f32 view)

            # norms: scale = rsqrt(ssq) = exp(-0.5*ln(ssq))
            sq_scr = cpool.tile([P, D], BF16, name=f"sqscr{r}", tag="sqscr")
            ssq_c = stats.tile([P, 1], F32, name=f"ssqc{r}", tag="ssqc")
            ssq_u = stats.tile([P, 1], F32, name=f"ssqu{r}", tag="ssqu")
            nc.vector.tensor_mul(sq_scr, csum, csum)
            nc.vector.tensor_reduce(ssq_c, sq_scr, axis=mybir.AxisListType.X,
                                    op=mybir.AluOpType.add)
            nc.vector.tensor_mul(sq_scr, u, u)
            nc.vector.tensor_reduce(ssq_u, sq_scr, axis=mybir.AxisListType.X,
                                    op=mybir.AluOpType.add)
            lc = stats.tile([P, 1], F32, name=f"lc{r}", tag="lc")
            lu = stats.tile([P, 1], F32, name=f"lu{r}", tag="lu")
            nc.scalar.activation(lc, ssq_c, AF.Ln)
            nc.scalar.activation(lu, ssq_u, AF.Ln)
            rc = stats.tile([P, 1], F32, name=f"rc{r}", tag="rc")
            ru = stats.tile([P, 1], F32, name=f"ru{r}", tag="ru")
            nc.scalar.activation(rc, lc, AF.Exp, scale=-0.5)
            nc.scalar.activation(ru, lu, AF.Exp, scale=-0.5)

            # normalize (f32 out so the PSUM->SBUF copies can ride ScalarE)
            chat = cpool.tile([P, D], F32, name=f"chat{r}", tag="chat")
            uhat = cpool.tile([P, D], F32, name=f"uhat{r}", tag="uhat")
            nc.vector.tensor_scalar_mul(chat, csum, rc)
            nc.vector.tensor_scalar_mul(uhat, u, ru)

            # diagonal cos (local)
            dg_scr = cpool.tile([P, D], F32, name=f"dgscr{r}", tag="dgscr")
            nc.vector.tensor_mul(dg_scr, chat, uhat)
            rd = nc.vector.tensor_reduce(diag_all[:, r:r + 1], dg_scr,
                                         axis=mybir.AxisListType.X,
                                         op=mybir.AluOpType.add)
            last_dve_b[0] = rd.ins

            # transposes on PE (f32 in -> f32 psum), cast to fp8/bf16 on ScalarE
            cT = cpool.tile([P, NT, P], FP8, name=f"cT{r}", tag="cT")
            for t in range(NT):
                pt = tpsum.tile([P, P], F32, name=f"ptc{r}_{t}", tag="ptc")
                ti = nc.tensor.transpose(pt, chat[:, t * P:(t + 1) * P], ident)
                last_transpose[0] = ti.ins
                nc.scalar.copy(cT[:, t, :], pt)
                pu = tpsum.tile([P, P], F32, name=f"ptu{r}_{t}", tag="ptu")
                ti = nc.tensor.transpose(pu, uhat[:, t * P:(t + 1) * P], ident)
                last_transpose[0] = ti.ins
                cp = nc.scalar.copy(uT[:, t, r * P:(r + 1) * P], pu)
                last_act_b[0] = cp.ins

            # bounce write (fp8). Rides the gpsimd SWDGE ring (otherwise
            # empty) so it is not FIFO-serialized behind the big X loads on
            # the sync ring.
            gi, slot = chunk_group[r]
            L = len(GROUPS[gi])
            bounces[gi] = bounces[gi] if bounces[gi] is not None else dram.tile(
                [L * P, NT * P], FP8, name=f"bounce_g{gi}")
            nc.gpsimd.dma_start(
                out=bounces[gi][slot * P:(slot + 1) * P, :],
                in_=cT.rearrange("p t i -> p (t i)"))
            if slot == L - 1:
                g = dram.tile([N_CORES * L * P, NT * P], FP8,
                              name=f"gath{gi}", addr_space="Shared")
                cc = nc.gpsimd.collective_compute(
                    "AllGather", mybir.AluOpType.bypass,
                    replica_groups=[list(range(N_CORES))],
                    ins=[bounces[gi].opt()], outs=[g.opt()],
                )
                cc_insts.append(cc.ins)
                gath.append(g)

        # ---------- phase C: gathered reads + matmuls + epilogue ----------
        # Ordering guards: phase-C work on DVE/ACT must sit behind all
        # phase-B work on those queues, so late chunks are never stalled
        # behind epilogue ops waiting on a gather.
        last_dve = [last_dve_b[0]]
        last_act = [last_act_b[0]]
        for gi, grp in enumerate(GROUPS):
            L = len(grp)
            g_sb = gpool.tile([P, N_CORES * L, NT, P], FP8, name=f"gsb{gi}",
                              tag=f"gsb{gi}")
            nc.sync.dma_start(
                out=g_sb,
                in_=gath[gi].rearrange("(c p) f -> p c f", p=P).rearrange(
                    "p c (t i) -> p c t i", t=NT))
            for q in range(NCHUNK):
                for slot in range(L):
                    ps = mpsum.tile([P, 2, N_CORES // 2 * P], F32,
                                    name=f"ps{gi}_{q}_{slot}", tag="ps")
                    for h in range(2):
                        for t in range(NT):
                            # rhs: ranks c in [4h,4h+4), chunk slot, d-tile t
                            cs = 4 * h * L + slot
                            rhs = g_sb[:, cs:cs + 3 * L + 1:L, t, :]
                            mm = nc.tensor.matmul(
                                ps[:, h, :],
                                uT[:, t, q * P:(q + 1) * P],
                                rhs,
                                start=(t == 0), stop=(t == NT - 1),
                            )
                            # keep every matmul behind all transposes in the
                            # PE queue so late-chunk transposes are never
                            # stalled by matmuls waiting on a gather.
                            if h == 0 and t == 0:
                                tile.add_dep_helper(
                                    mm.ins, last_transpose[0], sync=True,
                                    reason="PE: transposes before matmuls")
                    # epilogue: y = max(cos, eps); s += sum_k exp(w*y)
                    y = epool.tile([P, 2 * (N_CORES // 2) * P], BF16,
                                   name=f"y{gi}_{q}_{slot}", tag="y")
                    mx = nc.vector.tensor_scalar_max(
                        y, ps.rearrange("p a b -> p (a b)"), EPS)
                    e_scr = epool.tile([P, 2 * (N_CORES // 2) * P], BF16,
                                       name=f"escr{gi}_{q}_{slot}", tag="escr")
                    s_part = stats.tile([P, 1], F32, name=f"sp{gi}_{q}_{slot}",
                                        tag="sp")
                    ex = nc.scalar.activation(e_scr, y, AF.Exp, scale=w_val,
                                              accum_out=s_part)
                    nc.vector.tensor_add(s_acc[:, q:q + 1], s_acc[:, q:q + 1],
                                         s_part)

        # ---------- finals: ship s (exp-sums) and diag; host does the log
        nc.sync.dma_start(out=out[0].rearrange("(q p) -> p q", p=P), in_=s_acc)
        nc.sync.dma_start(out=out[1].rearrange("(q p) -> p q", p=P),
                          in_=diag_all)


_CACHE = {}


def kernel(dvecs, w, b):
    w_val = float(np.asarray(w))
    key = w_val
    if key not in _CACHE:
        _CACHE[key] = build_program(w_val)
    nc = _CACHE[key]
    dvecs = np.ascontiguousarray(np.asarray(dvecs, dtype=np.float32))
    in_maps = [
        {"dvecs": dvecs[c * LOCAL:(c + 1) * LOCAL]} for c in range(N_CORES)
    ]
    res = run_bass_kernel_spmd(nc, in_maps, core_ids=list(range(N_CORES)))
    total = 0.0
    for c in range(N_CORES):
        o = np.asarray(res.results[c]["out"], dtype=np.float64)
        s, diag = o[0], o[1]
        rows = np.log(s) - w_val * np.maximum(diag, EPS)
        total += float(rows.sum())
    return np.float32(total / N)


# revision 46
# speedup vs baseline: 1.0482x; 1.0482x over previous
"""AngProtoLoss (stable) distributed Bass kernel for 8 TRN2 NeuronCores.

Problem (reference):
    dvecs: (4096, 16, 512) f32
    centroids = mean(dvecs, axis=1)                  # (N, D)
    u = dvecs[:, -1, :]                              # (N, D)
    cos = clip(cos_sim(u, centroids), min=1e-6)      # (N, N)
    logits = cos * w + b
    loss = -mean(diag(log_softmax(logits)))
        = mean_i [ logsumexp_k(w*clip(cos_ik)) - w*clip(cos_ii) ]   (b cancels)

Sharding: data-parallel over speakers N. Each core gets 512 speakers (4
chunks of 128), computes local normalized centroids (bf16 tree sum ->
rsqrt-normalize), transposes them on the TensorE, all-gathers them in fp8
(one allgather per chunk, pipelined against the load/centroid phase), then
computes its 512 rows of the cos matrix in bf16 x fp8 matmuls, applies
clip+exp (with ScalarE accumulate) for the log-sum-exp, and the local
diagonal terms. Device outputs per-row exp-sums and diagonal cos; the host
unshard does rows = log(s) - w*clip(diag) and means over N (b cancels in
log-softmax exactly).

Schedule notes (engine queues are FIFO):
 - gpsimd queue holds only the bounce writes + collectives so each allgather
   triggers as soon as its bounce is written (never stuck behind big loads).
 - X loads and gathered reads ride the sync HWDGE ring in data-ready order.
 - explicit add_dep edges keep phase-C matmuls/epilogue behind all phase-B
   work on PE/DVE/ACT queues, so a late chunk's transposes are never stalled
   behind ops waiting on a gather.
 - bounce layout is [128 d-rows x 512B (t,i)-cols] so every DMA touching
   HBM moves >=512B contiguous runs (small descriptors starve during
   collectives).
"""

import os
import sys

for _p in ("/opt/trn_rl_repo",):
    if os.path.isdir(_p) and _p not in sys.path:
        sys.path.append(_p)

import numpy as np

import concourse.bass as bass
import concourse.tile as tile
from concourse import bacc, mybir
from concourse.bass_utils import run_bass_kernel_spmd
from concourse.masks import make_identity

N_CORES = 8
N, M, D = 4096, 16, 512
P = 128                     # partitions
LOCAL = N // N_CORES        # 512 speakers per core
NCHUNK = LOCAL // P         # 4 chunks of 128 speakers
NT = D // P                 # 4 d-tiles
EPS = 1e-6

F32 = mybir.dt.float32
BF16 = mybir.dt.bfloat16
FP8 = mybir.dt.float8e4
AF = mybir.ActivationFunctionType


def build_program(w_val: float):
    nc = bacc.Bacc("TRN2", target_bir_lowering=False, debug=False,
                   num_devices=N_CORES)
    dvecs = nc.dram_tensor("dvecs", [LOCAL, M, D], F32, kind="ExternalInput").ap()
    out = nc.dram_tensor("out", [2, LOCAL], F32, kind="ExternalOutput").ap()

    with tile.TileContext(nc) as tc:
        _build(nc, tc, dvecs, out, w_val)
    nc.compile()
    return nc


def _build(nc, tc, dvecs, out, w_val):
    from contextlib import ExitStack
    ctx = ExitStack()
    with ctx:
        singles = ctx.enter_context(tc.tile_pool(name="singles", bufs=1))
        xpool = ctx.enter_context(tc.tile_pool(name="xpool", bufs=2))
        tree = ctx.enter_context(tc.tile_pool(name="tree", bufs=2))
        cpool = ctx.enter_context(tc.tile_pool(name="cpool", bufs=2))
        stats = ctx.enter_context(tc.tile_pool(name="stats", bufs=4))
        gpool = ctx.enter_context(tc.tile_pool(name="gpool", bufs=1))
        epool = ctx.enter_context(tc.tile_pool(name="epool", bufs=3))
        tpsum = ctx.enter_context(tc.tile_pool(name="tpsum", bufs=2, space="PSUM"))
        mpsum = ctx.enter_context(tc.tile_pool(name="mpsum", bufs=2, space="PSUM"))
        dram = ctx.enter_context(tc.tile_pool(name="dram", bufs=1, space="DRAM"))

        ident = singles.tile([P, P], F32)
        make_identity(nc, ident)

        # persistent across the whole kernel
        uT = singles.tile([P, NT, LOCAL], BF16)          # u^T: [d_in_tile, t, i]
        s_acc = singles.tile([P, NCHUNK], F32)           # sum_k exp(w*clip(cos))
        diag_all = singles.tile([P, NCHUNK], F32)        # diag cos, per q
        nc.vector.memset(s_acc, 0.0)

        # ---------- phase A: loads first (sync ring order) ----------
        xs = []
        for r in range(NCHUNK):
            x = xpool.tile([P, M, D], F32, name=f"x{r}", tag="x")
            nc.sync.dma_start(out=x, in_=dvecs[r * P:(r + 1) * P, :, :])
            xs.append(x)

        # ---------- phase B: per-chunk centroid pipeline + allgather ----------
        # One allgather per chunk (grouping chunks into fewer, bigger AGs
        # measured slower: the 1MB fp8 AG falls into the slow RDH regime).
        GROUPS = [[0], [1], [2], [3]]
        chunk_group = {}
        for gi, grp in enumerate(GROUPS):
            for slot, rr in enumerate(grp):
                chunk_group[rr] = (gi, slot)
        bounces = [None] * len(GROUPS)
        cc_insts = []
        gath = []
        last_transpose = [None]
        last_dve_b = [None]
        last_act_b = [None]
        # ssq/scale slots for all chunks: [:, r, 0] = centroid, [:, r, 1] = u.
        # Norm transcendentals (Ln then Exp) run batched per chunk PAIR so the
        # ACT table set switches ~4x per kernel instead of 14x -- each switch
        # is a 1.5us TDRAM DMA that lands inside the collective windows.
        ssq_all = singles.tile([P, NCHUNK, 2], F32)
        scales_all = singles.tile([P, NCHUNK, 2], F32)
        csums = []
        u_saves = []
        for r in range(NCHUNK):
            x = xs[r]
            # centroid sum over m: first level casts f32 -> bf16
            t1 = tree.tile([P, M // 2, D], BF16, name=f"t1_{r}", tag="t1")
            for j in range(M // 2):
                nc.vector.tensor_add(t1[:, j, :], x[:, 2 * j, :], x[:, 2 * j + 1, :])
            t2 = tree.tile([P, M // 4, D], BF16, name=f"t2_{r}", tag="t2")
            for j in range(M // 4):
                nc.vector.tensor_add(t2[:, j, :], t1[:, 2 * j, :], t1[:, 2 * j + 1, :])
            t3 = tree.tile([P, M // 8, D], BF16, name=f"t3_{r}", tag="t3")
            for j in range(M // 8):
                nc.vector.tensor_add(t3[:, j, :], t2[:, 2 * j, :], t2[:, 2 * j + 1, :])
            csum = cpool.tile([P, D], BF16, name=f"csum{r}", tag="csum")
            nc.vector.tensor_add(csum, t3[:, 0, :], t3[:, 1, :])
            csums.append(csum)

            # save the last utterance (frees the big X tile early)
            u_save = cpool.tile([P, D], BF16, name=f"usave{r}", tag="usave")
            nc.vector.tensor_copy(u_save, x[:, M - 1, :])
            u_saves.append(u_save)

            sq_scr = cpool.tile([P, D], BF16, name=f"sqscr{r}", tag="sqscr")
            nc.vector.tensor_mul(sq_scr, csum, csum)
            nc.vector.tensor_reduce(ssq_all[:, r, 0:1], sq_scr,
                                    axis=mybir.AxisListType.X,
                                    op=mybir.AluOpType.add)
            nc.vector.tensor_mul(sq_scr, x[:, M - 1, :], x[:, M - 1, :])
            nc.vector.tensor_reduce(ssq_all[:, r, 1:2], sq_scr,
                                    axis=mybir.AxisListType.X,
                                    op=mybir.AluOpType.add)

            if r % 2 == 0:
                continue
            # ---- batched norms + downstream for the pair (r-1, r) ----
            p0 = r - 1
            ln_scr = stats.tile([P, 2, 2], F32, name=f"ln{r}", tag="ln")
            nc.scalar.activation(ln_scr, ssq_all[:, p0:r + 1, :], AF.Ln)
            nc.scalar.activation(scales_all[:, p0:r + 1, :], ln_scr,
                                 AF.Exp, scale=-0.5)
            for rr in (p0, r):
                # normalize (f32 out: PSUM->SBUF copies ride ScalarE)
                chat = cpool.tile([P, D], F32, name=f"chat{rr}", tag="chat")
                uhat = cpool.tile([P, D], F32, name=f"uhat{rr}", tag="uhat")
                nc.vector.tensor_scalar_mul(chat, csums[rr],
                                            scales_all[:, rr, 0:1])
                nc.vector.tensor_scalar_mul(uhat, u_saves[rr],
                                            scales_all[:, rr, 1:2])

                # diagonal cos (local)
                dg_scr = cpool.tile([P, D], F32, name=f"dgscr{rr}", tag="dgscr")
                nc.vector.tensor_mul(dg_scr, chat, uhat)
                rd = nc.vector.tensor_reduce(diag_all[:, rr:rr + 1], dg_scr,
                                             axis=mybir.AxisListType.X,
                                             op=mybir.AluOpType.add)
                last_dve_b[0] = rd.ins

                # transposes on PE (f32 -> f32 psum), cast to fp8/bf16 on ACT
                cT = cpool.tile([P, NT, P], FP8, name=f"cT{rr}", tag="cT")
                for t in range(NT):
                    pt = tpsum.tile([P, P], F32, name=f"ptc{rr}_{t}", tag="ptc")
                    ti = nc.tensor.transpose(pt, chat[:, t * P:(t + 1) * P],
                                             ident)
                    last_transpose[0] = ti.ins
                    nc.scalar.copy(cT[:, t, :], pt)
                    pu = tpsum.tile([P, P], F32, name=f"ptu{rr}_{t}", tag="ptu")
                    ti = nc.tensor.transpose(pu, uhat[:, t * P:(t + 1) * P],
                                             ident)
                    last_transpose[0] = ti.ins
                    cp = nc.scalar.copy(uT[:, t, rr * P:(rr + 1) * P], pu)
                    last_act_b[0] = cp.ins

                # bounce write (fp8) on the gpsimd SWDGE ring (otherwise
                # empty) so it is not FIFO-serialized behind the X loads.
                gi, slot = chunk_group[rr]
                L = len(GROUPS[gi])
                bounces[gi] = bounces[gi] if bounces[gi] is not None else \
                    dram.tile([L * P, NT * P], FP8, name=f"bounce_g{gi}")
                nc.gpsimd.dma_start(
                    out=bounces[gi][slot * P:(slot + 1) * P, :],
                    in_=cT.rearrange("p t i -> p (t i)"))
                if slot == L - 1:
                    g = dram.tile([N_CORES * L * P, NT * P], FP8,
                                  name=f"gath{gi}", addr_space="Shared")
                    cc = nc.gpsimd.collective_compute(
                        "AllGather", mybir.AluOpType.bypass,
                        replica_groups=[list(range(N_CORES))],
                        ins=[bounces[gi].opt()], outs=[g.opt()],
                    )
                    cc_insts.append(cc.ins)
                    gath.append(g)

        # ---------- phase C: gathered reads + matmuls + epilogue ----------
        # Ordering guards: phase-C work on DVE/ACT must sit behind all
        # phase-B work on those queues, so late chunks are never stalled
        # behind epilogue ops waiting on a gather.
        last_dve = [last_dve_b[0]]
        last_act = [last_act_b[0]]
        for gi, grp in enumerate(GROUPS):
            L = len(grp)
            g_sb = gpool.tile([P, N_CORES * L, NT, P], FP8, name=f"gsb{gi}",
                              tag=f"gsb{gi}")
            nc.sync.dma_start(
                out=g_sb,
                in_=gath[gi].rearrange("(c p) f -> p c f", p=P).rearrange(
                    "p c (t i) -> p c t i", t=NT))
            for q in range(NCHUNK):
                for slot in range(L):
                    ps = mpsum.tile([P, 2, N_CORES // 2 * P], F32,
                                    name=f"ps{gi}_{q}_{slot}", tag="ps")
                    for h in range(2):
                        for t in range(NT):
                            # rhs: ranks c in [4h,4h+4), chunk slot, d-tile t
                            cs = 4 * h * L + slot
                            rhs = g_sb[:, cs:cs + 3 * L + 1:L, t, :]
                            mm = nc.tensor.matmul(
                                ps[:, h, :],
                                uT[:, t, q * P:(q + 1) * P],
                                rhs,
                                start=(t == 0), stop=(t == NT - 1),
                            )
                            # keep every matmul behind all transposes in the
                            # PE queue so late-chunk transposes are never
                            # stalled by matmuls waiting on a gather.
                            if h == 0 and t == 0:
                                tile.add_dep_helper(
                                    mm.ins, last_transpose[0], sync=True,
                                    reason="PE: transposes before matmuls")
                    # epilogue: y = max(cos, eps); s += sum_k exp(w*y)
                    y = epool.tile([P, 2 * (N_CORES // 2) * P], BF16,
                                   name=f"y{gi}_{q}_{slot}", tag="y")
                    mx = nc.vector.tensor_scalar_max(
                        y, ps.rearrange("p a b -> p (a b)"), EPS)
                    e_scr = epool.tile([P, 2 * (N_CORES // 2) * P], BF16,
                                       name=f"escr{gi}_{q}_{slot}", tag="escr")
                    s_part = stats.tile([P, 1], F32, name=f"sp{gi}_{q}_{slot}",
                                        tag="sp")
                    ex = nc.scalar.activation(e_scr, y, AF.Exp, scale=w_val,
                                              accum_out=s_part)
                    nc.vector.tensor_add(s_acc[:, q:q + 1], s_acc[:, q:q + 1],
                                         s_part)

        # ---------- finals: ship s (exp-sums) and diag; host does the log
        nc.sync.dma_start(out=out[0].rearrange("(q p) -> p q", p=P), in_=s_acc)
        nc.sync.dma_start(out=out[1].rearrange("(q p) -> p q", p=P),
                          in_=diag_all)


_CACHE = {}


def kernel(dvecs, w, b):
    w_val = float(np.asarray(w))
    key = w_val
    if key not in _CACHE:
        _CACHE[key] = build_program(w_val)
    nc = _CACHE[key]
    dvecs = np.ascontiguousarray(np.asarray(dvecs, dtype=np.float32))
    in_maps = [
        {"dvecs": dvecs[c * LOCAL:(c + 1) * LOCAL]} for c in range(N_CORES)
    ]
    res = run_bass_kernel_spmd(nc, in_maps, core_ids=list(range(N_CORES)))
    total = 0.0
    for c in range(N_CORES):
        o = np.asarray(res.results[c]["out"], dtype=np.float64)
        s, diag = o[0], o[1]
        rows = np.log(s) - w_val * np.maximum(diag, EPS)
        total += float(rows.sum())
    return np.float32(total / N)


# revision 47
# speedup vs baseline: 1.0523x; 1.0039x over previous
"""AngProtoLoss (stable) distributed Bass kernel for 8 TRN2 NeuronCores.

Problem (reference):
    dvecs: (4096, 16, 512) f32
    centroids = mean(dvecs, axis=1)                  # (N, D)
    u = dvecs[:, -1, :]                              # (N, D)
    cos = clip(cos_sim(u, centroids), min=1e-6)      # (N, N)
    logits = cos * w + b
    loss = -mean(diag(log_softmax(logits)))
        = mean_i [ logsumexp_k(w*clip(cos_ik)) - w*clip(cos_ii) ]   (b cancels)

Sharding: data-parallel over speakers N. Each core gets 512 speakers (4
chunks of 128), computes local normalized centroids (bf16 tree sum ->
rsqrt-normalize), transposes them on the TensorE, all-gathers them in fp8
(one allgather per chunk, pipelined against the load/centroid phase), then
computes its 512 rows of the cos matrix in bf16 x fp8 matmuls, applies
clip+exp (with ScalarE accumulate) for the log-sum-exp, and the local
diagonal terms. Device outputs per-row exp-sums and diagonal cos; the host
unshard does rows = log(s) - w*clip(diag) and means over N (b cancels in
log-softmax exactly).

Schedule notes (engine queues are FIFO):
 - gpsimd queue holds only the bounce writes + collectives so each allgather
   triggers as soon as its bounce is written (never stuck behind big loads).
 - X loads and gathered reads ride the sync HWDGE ring in data-ready order.
 - explicit add_dep edges keep phase-C matmuls/epilogue behind all phase-B
   work on PE/DVE/ACT queues, so a late chunk's transposes are never stalled
   behind ops waiting on a gather.
 - bounce layout is [128 d-rows x 512B (t,i)-cols] so every DMA touching
   HBM moves >=512B contiguous runs (small descriptors starve during
   collectives).
"""

import os
import sys

for _p in ("/opt/trn_rl_repo",):
    if os.path.isdir(_p) and _p not in sys.path:
        sys.path.append(_p)

import numpy as np

import concourse.bass as bass
import concourse.tile as tile
from concourse import bacc, mybir
from concourse.bass_utils import run_bass_kernel_spmd
from concourse.masks import make_identity

N_CORES = 8
N, M, D = 4096, 16, 512
P = 128                     # partitions
LOCAL = N // N_CORES        # 512 speakers per core
NCHUNK = LOCAL // P         # 4 chunks of 128 speakers
NT = D // P                 # 4 d-tiles
EPS = 1e-6

F32 = mybir.dt.float32
BF16 = mybir.dt.bfloat16
FP8 = mybir.dt.float8e4
AF = mybir.ActivationFunctionType


def build_program(w_val: float):
    nc = bacc.Bacc("TRN2", target_bir_lowering=False, debug=False,
                   num_devices=N_CORES)
    dvecs = nc.dram_tensor("dvecs", [LOCAL, M, D], F32, kind="ExternalInput").ap()
    out = nc.dram_tensor("out", [2, LOCAL], F32, kind="ExternalOutput").ap()

    with tile.TileContext(nc) as tc:
        _build(nc, tc, dvecs, out, w_val)
    nc.compile()
    return nc


def _build(nc, tc, dvecs, out, w_val):
    from contextlib import ExitStack
    ctx = ExitStack()
    with ctx:
        singles = ctx.enter_context(tc.tile_pool(name="singles", bufs=1))
        xpool = ctx.enter_context(tc.tile_pool(name="xpool", bufs=2))
        tree = ctx.enter_context(tc.tile_pool(name="tree", bufs=2))
        cpool = ctx.enter_context(tc.tile_pool(name="cpool", bufs=2))
        stats = ctx.enter_context(tc.tile_pool(name="stats", bufs=4))
        gpool = ctx.enter_context(tc.tile_pool(name="gpool", bufs=1))
        epool = ctx.enter_context(tc.tile_pool(name="epool", bufs=3))
        tpsum = ctx.enter_context(tc.tile_pool(name="tpsum", bufs=2, space="PSUM"))
        mpsum = ctx.enter_context(tc.tile_pool(name="mpsum", bufs=3, space="PSUM"))
        dram = ctx.enter_context(tc.tile_pool(name="dram", bufs=1, space="DRAM"))

        ident = singles.tile([P, P], F32)
        make_identity(nc, ident)

        # persistent across the whole kernel
        uT = singles.tile([P, NT, LOCAL], BF16)          # u^T: [d_in_tile, t, i]
        s_acc = singles.tile([P, NCHUNK], F32)           # sum_k exp(w*clip(cos))
        diag_all = singles.tile([P, NCHUNK], F32)        # diag cos, per q
        nc.vector.memset(s_acc, 0.0)

        # ---------- phase A: loads first (sync ring order) ----------
        xs = []
        for r in range(NCHUNK):
            x = xpool.tile([P, M, D], F32, name=f"x{r}", tag="x")
            nc.sync.dma_start(out=x, in_=dvecs[r * P:(r + 1) * P, :, :])
            xs.append(x)

        # ---------- phase B: per-chunk centroid pipeline + allgather ----------
        # One allgather per chunk (grouping chunks into fewer, bigger AGs
        # measured slower: the 1MB fp8 AG falls into the slow RDH regime).
        GROUPS = [[0], [1], [2], [3]]
        chunk_group = {}
        for gi, grp in enumerate(GROUPS):
            for slot, rr in enumerate(grp):
                chunk_group[rr] = (gi, slot)
        bounces = [None] * len(GROUPS)
        cc_insts = []
        gath = []
        last_transpose = [None]
        last_dve_b = [None]
        last_act_b = [None]
        # ssq/scale slots for all chunks: [:, r, 0] = centroid, [:, r, 1] = u.
        # Norm transcendentals (Ln then Exp) run batched per chunk PAIR so the
        # ACT table set switches ~4x per kernel instead of 14x -- each switch
        # is a 1.5us TDRAM DMA that lands inside the collective windows.
        ssq_all = singles.tile([P, NCHUNK, 2], F32)
        scales_all = singles.tile([P, NCHUNK, 2], F32)
        csums = []
        u_saves = []
        for r in range(NCHUNK):
            x = xs[r]
            # centroid sum over m: first level casts f32 -> bf16
            t1 = tree.tile([P, M // 2, D], BF16, name=f"t1_{r}", tag="t1")
            for j in range(M // 2):
                nc.vector.tensor_add(t1[:, j, :], x[:, 2 * j, :], x[:, 2 * j + 1, :])
            t2 = tree.tile([P, M // 4, D], BF16, name=f"t2_{r}", tag="t2")
            for j in range(M // 4):
                nc.vector.tensor_add(t2[:, j, :], t1[:, 2 * j, :], t1[:, 2 * j + 1, :])
            t3 = tree.tile([P, M // 8, D], BF16, name=f"t3_{r}", tag="t3")
            for j in range(M // 8):
                nc.vector.tensor_add(t3[:, j, :], t2[:, 2 * j, :], t2[:, 2 * j + 1, :])
            csum = cpool.tile([P, D], BF16, name=f"csum{r}", tag="csum")
            nc.vector.tensor_add(csum, t3[:, 0, :], t3[:, 1, :])
            csums.append(csum)

            # save the last utterance (frees the big X tile early)
            u_save = cpool.tile([P, D], BF16, name=f"usave{r}", tag="usave")
            nc.vector.tensor_copy(u_save, x[:, M - 1, :])
            u_saves.append(u_save)

            sq_scr = cpool.tile([P, D], BF16, name=f"sqscr{r}", tag="sqscr")
            nc.vector.tensor_mul(sq_scr, csum, csum)
            nc.vector.tensor_reduce(ssq_all[:, r, 0:1], sq_scr,
                                    axis=mybir.AxisListType.X,
                                    op=mybir.AluOpType.add)
            nc.vector.tensor_mul(sq_scr, x[:, M - 1, :], x[:, M - 1, :])
            nc.vector.tensor_reduce(ssq_all[:, r, 1:2], sq_scr,
                                    axis=mybir.AxisListType.X,
                                    op=mybir.AluOpType.add)

            if r % 2 == 0:
                continue
            # ---- batched norms + downstream for the pair (r-1, r) ----
            p0 = r - 1
            ln_scr = stats.tile([P, 2, 2], F32, name=f"ln{r}", tag="ln")
            nc.scalar.activation(ln_scr, ssq_all[:, p0:r + 1, :], AF.Ln)
            nc.scalar.activation(scales_all[:, p0:r + 1, :], ln_scr,
                                 AF.Exp, scale=-0.5)
            for rr in (p0, r):
                # normalize (f32 out: PSUM->SBUF copies ride ScalarE)
                chat = cpool.tile([P, D], F32, name=f"chat{rr}", tag="chat")
                uhat = cpool.tile([P, D], F32, name=f"uhat{rr}", tag="uhat")
                nc.vector.tensor_scalar_mul(chat, csums[rr],
                                            scales_all[:, rr, 0:1])
                nc.vector.tensor_scalar_mul(uhat, u_saves[rr],
                                            scales_all[:, rr, 1:2])

                # diagonal cos (local)
                dg_scr = cpool.tile([P, D], F32, name=f"dgscr{rr}", tag="dgscr")
                nc.vector.tensor_mul(dg_scr, chat, uhat)
                rd = nc.vector.tensor_reduce(diag_all[:, rr:rr + 1], dg_scr,
                                             axis=mybir.AxisListType.X,
                                             op=mybir.AluOpType.add)
                last_dve_b[0] = rd.ins

                # transposes on PE (f32 -> f32 psum), cast to fp8/bf16 on ACT
                cT = cpool.tile([P, NT, P], FP8, name=f"cT{rr}", tag="cT")
                for t in range(NT):
                    pt = tpsum.tile([P, P], F32, name=f"ptc{rr}_{t}", tag="pt")
                    ti = nc.tensor.transpose(pt, chat[:, t * P:(t + 1) * P],
                                             ident)
                    last_transpose[0] = ti.ins
                    nc.scalar.copy(cT[:, t, :], pt)
                    pu = tpsum.tile([P, P], F32, name=f"ptu{rr}_{t}", tag="pt")
                    ti = nc.tensor.transpose(pu, uhat[:, t * P:(t + 1) * P],
                                             ident)
                    last_transpose[0] = ti.ins
                    cp = nc.scalar.copy(uT[:, t, rr * P:(rr + 1) * P], pu)
                    last_act_b[0] = cp.ins

                # bounce write (fp8) on the gpsimd SWDGE ring (otherwise
                # empty) so it is not FIFO-serialized behind the X loads.
                gi, slot = chunk_group[rr]
                L = len(GROUPS[gi])
                bounces[gi] = bounces[gi] if bounces[gi] is not None else \
                    dram.tile([L * P, NT * P], FP8, name=f"bounce_g{gi}")
                nc.gpsimd.dma_start(
                    out=bounces[gi][slot * P:(slot + 1) * P, :],
                    in_=cT.rearrange("p t i -> p (t i)"))
                if slot == L - 1:
                    g = dram.tile([N_CORES * L * P, NT * P], FP8,
                                  name=f"gath{gi}", addr_space="Shared")
                    cc = nc.gpsimd.collective_compute(
                        "AllGather", mybir.AluOpType.bypass,
                        replica_groups=[list(range(N_CORES))],
                        ins=[bounces[gi].opt()], outs=[g.opt()],
                    )
                    cc_insts.append(cc.ins)
                    gath.append(g)

        # ---------- phase C: gathered reads + matmuls + epilogue ----------
        # Ordering guards: phase-C work on DVE/ACT must sit behind all
        # phase-B work on those queues, so late chunks are never stalled
        # behind epilogue ops waiting on a gather.
        last_dve = [last_dve_b[0]]
        last_act = [last_act_b[0]]
        for gi, grp in enumerate(GROUPS):
            L = len(grp)
            g_sb = gpool.tile([P, N_CORES * L, NT, P], FP8, name=f"gsb{gi}",
                              tag=f"gsb{gi}")
            nc.sync.dma_start(
                out=g_sb,
                in_=gath[gi].rearrange("(c p) f -> p c f", p=P).rearrange(
                    "p c (t i) -> p c t i", t=NT))
            for q in range(NCHUNK):
                for slot in range(L):
                    ps = mpsum.tile([P, 2, N_CORES // 2 * P], F32,
                                    name=f"ps{gi}_{q}_{slot}", tag="ps")
                    for h in range(2):
                        for t in range(NT):
                            # rhs: ranks c in [4h,4h+4), chunk slot, d-tile t
                            cs = 4 * h * L + slot
                            rhs = g_sb[:, cs:cs + 3 * L + 1:L, t, :]
                            mm = nc.tensor.matmul(
                                ps[:, h, :],
                                uT[:, t, q * P:(q + 1) * P],
                                rhs,
                                start=(t == 0), stop=(t == NT - 1),
                            )
                            # keep every matmul behind all transposes in the
                            # PE queue so late-chunk transposes are never
                            # stalled by matmuls waiting on a gather.
                            if h == 0 and t == 0:
                                tile.add_dep_helper(
                                    mm.ins, last_transpose[0], sync=True,
                                    reason="PE: transposes before matmuls")
                    # epilogue: y = max(cos, eps); s += sum_k exp(w*y)
                    y = epool.tile([P, 2 * (N_CORES // 2) * P], BF16,
                                   name=f"y{gi}_{q}_{slot}", tag="y")
                    mx = nc.vector.tensor_scalar_max(
                        y, ps.rearrange("p a b -> p (a b)"), EPS)
                    e_scr = epool.tile([P, 2 * (N_CORES // 2) * P], BF16,
                                       name=f"escr{gi}_{q}_{slot}", tag="escr")
                    s_part = stats.tile([P, 1], F32, name=f"sp{gi}_{q}_{slot}",
                                        tag="sp")
                    ex = nc.scalar.activation(e_scr, y, AF.Exp, scale=w_val,
                                              accum_out=s_part)
                    nc.vector.tensor_add(s_acc[:, q:q + 1], s_acc[:, q:q + 1],
                                         s_part)

        # ---------- finals: ship s (exp-sums) and diag; host does the log
        nc.sync.dma_start(out=out[0].rearrange("(q p) -> p q", p=P), in_=s_acc)
        nc.sync.dma_start(out=out[1].rearrange("(q p) -> p q", p=P),
                          in_=diag_all)


_CACHE = {}


def kernel(dvecs, w, b):
    w_val = float(np.asarray(w))
    key = w_val
    if key not in _CACHE:
        _CACHE[key] = build_program(w_val)
    nc = _CACHE[key]
    dvecs = np.ascontiguousarray(np.asarray(dvecs, dtype=np.float32))
    in_maps = [
        {"dvecs": dvecs[c * LOCAL:(c + 1) * LOCAL]} for c in range(N_CORES)
    ]
    res = run_bass_kernel_spmd(nc, in_maps, core_ids=list(range(N_CORES)))
    total = 0.0
    for c in range(N_CORES):
        o = np.asarray(res.results[c]["out"], dtype=np.float64)
        s, diag = o[0], o[1]
        rows = np.log(s) - w_val * np.maximum(diag, EPS)
        total += float(rows.sum())
    return np.float32(total / N)


# revision 48
# speedup vs baseline: 1.0682x; 1.0150x over previous
"""AngProtoLoss (stable) distributed Bass kernel for 8 TRN2 NeuronCores.

Problem (reference):
    dvecs: (4096, 16, 512) f32
    centroids = mean(dvecs, axis=1)                  # (N, D)
    u = dvecs[:, -1, :]                              # (N, D)
    cos = clip(cos_sim(u, centroids), min=1e-6)      # (N, N)
    logits = cos * w + b
    loss = -mean(diag(log_softmax(logits)))
        = mean_i [ logsumexp_k(w*clip(cos_ik)) - w*clip(cos_ii) ]   (b cancels)

Sharding: data-parallel over speakers N. Each core gets 512 speakers (4
chunks of 128), computes local normalized centroids (bf16 tree sum ->
rsqrt-normalize), transposes them on the TensorE, all-gathers them in fp8
(one allgather per chunk, pipelined against the load/centroid phase), then
computes its 512 rows of the cos matrix in bf16 x fp8 matmuls, applies
clip+exp (with ScalarE accumulate) for the log-sum-exp, and the local
diagonal terms. Device outputs per-row exp-sums and diagonal cos; the host
unshard does rows = log(s) - w*clip(diag) and means over N (b cancels in
log-softmax exactly).

Schedule notes (engine queues are FIFO):
 - gpsimd queue holds only the bounce writes + collectives so each allgather
   triggers as soon as its bounce is written (never stuck behind big loads).
 - X loads and gathered reads ride the sync HWDGE ring in data-ready order.
 - explicit add_dep edges keep phase-C matmuls/epilogue behind all phase-B
   work on PE/DVE/ACT queues, so a late chunk's transposes are never stalled
   behind ops waiting on a gather.
 - bounce layout is [128 d-rows x 512B (t,i)-cols] so every DMA touching
   HBM moves >=512B contiguous runs (small descriptors starve during
   collectives).
"""

import os
import sys

for _p in ("/opt/trn_rl_repo",):
    if os.path.isdir(_p) and _p not in sys.path:
        sys.path.append(_p)

import numpy as np

import concourse.bass as bass
import concourse.tile as tile
from concourse import bacc, mybir
from concourse.bass_utils import run_bass_kernel_spmd
from concourse.masks import make_identity

N_CORES = 8
N, M, D = 4096, 16, 512
P = 128                     # partitions
LOCAL = N // N_CORES        # 512 speakers per core
NCHUNK = LOCAL // P         # 4 chunks of 128 speakers
NT = D // P                 # 4 d-tiles
EPS = 1e-6

F32 = mybir.dt.float32
BF16 = mybir.dt.bfloat16
FP8 = mybir.dt.float8e4
AF = mybir.ActivationFunctionType


def build_program(w_val: float):
    nc = bacc.Bacc("TRN2", target_bir_lowering=False, debug=False,
                   num_devices=N_CORES)
    dvecs = nc.dram_tensor("dvecs", [LOCAL, M, D], F32, kind="ExternalInput").ap()
    out = nc.dram_tensor("out", [2, LOCAL], F32, kind="ExternalOutput").ap()

    with tile.TileContext(nc) as tc:
        _build(nc, tc, dvecs, out, w_val)
    nc.compile()
    return nc


def _build(nc, tc, dvecs, out, w_val):
    from contextlib import ExitStack
    ctx = ExitStack()
    with ctx:
        singles = ctx.enter_context(tc.tile_pool(name="singles", bufs=1))
        xpool = ctx.enter_context(tc.tile_pool(name="xpool", bufs=2))
        tree = ctx.enter_context(tc.tile_pool(name="tree", bufs=2))
        cpool = ctx.enter_context(tc.tile_pool(name="cpool", bufs=2))
        stats = ctx.enter_context(tc.tile_pool(name="stats", bufs=4))
        gpool = ctx.enter_context(tc.tile_pool(name="gpool", bufs=1))
        epool = ctx.enter_context(tc.tile_pool(name="epool", bufs=3))
        tpsum = ctx.enter_context(tc.tile_pool(name="tpsum", bufs=2, space="PSUM"))
        mpsum = ctx.enter_context(tc.tile_pool(name="mpsum", bufs=3, space="PSUM"))
        dram = ctx.enter_context(tc.tile_pool(name="dram", bufs=1, space="DRAM"))

        ident = singles.tile([P, P], F32)
        make_identity(nc, ident)

        # persistent across the whole kernel
        uT = singles.tile([P, NT, LOCAL], BF16)          # u^T: [d_in_tile, t, i]
        s_acc = singles.tile([P, NCHUNK], F32)           # sum_k exp(w*clip(cos))
        diag_all = singles.tile([P, NCHUNK], F32)        # diag cos, per q
        nc.vector.memset(s_acc, 0.0)

        # ---------- phase A: loads first (sync ring order) ----------
        xs = []
        for r in range(NCHUNK):
            x = xpool.tile([P, M, D], F32, name=f"x{r}", tag="x")
            nc.sync.dma_start(out=x, in_=dvecs[r * P:(r + 1) * P, :, :])
            xs.append(x)

        # ---------- phase B: per-chunk centroid pipeline + allgather ----------
        # One allgather per chunk (grouping chunks into fewer, bigger AGs
        # measured slower: the 1MB fp8 AG falls into the slow RDH regime).
        GROUPS = [[0], [1], [2], [3]]
        chunk_group = {}
        for gi, grp in enumerate(GROUPS):
            for slot, rr in enumerate(grp):
                chunk_group[rr] = (gi, slot)
        bounces = [None] * len(GROUPS)
        cc_insts = []
        gath = []
        last_transpose = [None]
        last_dve_b = [None]
        last_act_b = [None]
        # ssq/scale slots for all chunks: [:, r, 0] = centroid, [:, r, 1] = u.
        # Norm transcendentals (Ln then Exp) run batched per chunk PAIR so the
        # ACT table set switches ~4x per kernel instead of 14x -- each switch
        # is a 1.5us TDRAM DMA that lands inside the collective windows.
        ssq_all = singles.tile([P, NCHUNK, 2], F32)
        scales_all = singles.tile([P, NCHUNK, 2], F32)
        csums = []
        u_saves = []
        for r in range(NCHUNK):
            x = xs[r]
            # centroid sum over m: first level casts f32 -> bf16
            t1 = tree.tile([P, M // 2, D], BF16, name=f"t1_{r}", tag="t1")
            for j in range(M // 2):
                nc.vector.tensor_add(t1[:, j, :], x[:, 2 * j, :], x[:, 2 * j + 1, :])
            t2 = tree.tile([P, M // 4, D], BF16, name=f"t2_{r}", tag="t2")
            for j in range(M // 4):
                nc.vector.tensor_add(t2[:, j, :], t1[:, 2 * j, :], t1[:, 2 * j + 1, :])
            t3 = tree.tile([P, M // 8, D], BF16, name=f"t3_{r}", tag="t3")
            for j in range(M // 8):
                nc.vector.tensor_add(t3[:, j, :], t2[:, 2 * j, :], t2[:, 2 * j + 1, :])
            csum = cpool.tile([P, D], BF16, name=f"csum{r}", tag="csum")
            nc.vector.tensor_add(csum, t3[:, 0, :], t3[:, 1, :])
            csums.append(csum)

            # save the last utterance (frees the big X tile early)
            u_save = cpool.tile([P, D], BF16, name=f"usave{r}", tag="usave")
            nc.vector.tensor_copy(u_save, x[:, M - 1, :])
            u_saves.append(u_save)

            sq_scr = cpool.tile([P, D], BF16, name=f"sqscr{r}", tag="sqscr")
            nc.vector.tensor_mul(sq_scr, csum, csum)
            nc.vector.tensor_reduce(ssq_all[:, r, 0:1], sq_scr,
                                    axis=mybir.AxisListType.X,
                                    op=mybir.AluOpType.add)
            nc.vector.tensor_mul(sq_scr, x[:, M - 1, :], x[:, M - 1, :])
            nc.vector.tensor_reduce(ssq_all[:, r, 1:2], sq_scr,
                                    axis=mybir.AxisListType.X,
                                    op=mybir.AluOpType.add)

            if r % 2 == 0:
                continue
            # ---- batched norms + downstream for the pair (r-1, r) ----
            p0 = r - 1
            ln_scr = stats.tile([P, 2, 2], F32, name=f"ln{r}", tag="ln")
            nc.scalar.activation(ln_scr, ssq_all[:, p0:r + 1, :], AF.Ln)
            nc.scalar.activation(scales_all[:, p0:r + 1, :], ln_scr,
                                 AF.Exp, scale=-0.5)
            for rr in (p0, r):
                # normalize (f32 out: PSUM->SBUF copies ride ScalarE)
                chat = cpool.tile([P, D], F32, name=f"chat{rr}", tag="chat")
                uhat = cpool.tile([P, D], F32, name=f"uhat{rr}", tag="uhat")
                nc.vector.tensor_scalar_mul(chat, csums[rr],
                                            scales_all[:, rr, 0:1])
                nc.vector.tensor_scalar_mul(uhat, u_saves[rr],
                                            scales_all[:, rr, 1:2])

                # diagonal cos (local)
                dg_scr = cpool.tile([P, D], F32, name=f"dgscr{rr}", tag="dgscr")
                nc.vector.tensor_mul(dg_scr, chat, uhat)
                rd = nc.vector.tensor_reduce(diag_all[:, rr:rr + 1], dg_scr,
                                             axis=mybir.AxisListType.X,
                                             op=mybir.AluOpType.add)
                last_dve_b[0] = rd.ins

                # transposes on PE (f32 -> f32 psum), cast to fp8/bf16 on ACT
                cT = cpool.tile([P, NT, P], FP8, name=f"cT{rr}", tag="cT")
                for t in range(NT):
                    pt = tpsum.tile([P, P], F32, name=f"ptc{rr}_{t}", tag="pt")
                    ti = nc.tensor.transpose(pt, chat[:, t * P:(t + 1) * P],
                                             ident)
                    last_transpose[0] = ti.ins
                    nc.scalar.copy(cT[:, t, :], pt)
                    pu = tpsum.tile([P, P], F32, name=f"ptu{rr}_{t}", tag="pt")
                    ti = nc.tensor.transpose(pu, uhat[:, t * P:(t + 1) * P],
                                             ident)
                    last_transpose[0] = ti.ins
                    cp = nc.scalar.copy(uT[:, t, rr * P:(rr + 1) * P], pu)
                    last_act_b[0] = cp.ins

                # bounce write (fp8) on the gpsimd SWDGE ring (otherwise
                # empty) so it is not FIFO-serialized behind the X loads.
                gi, slot = chunk_group[rr]
                L = len(GROUPS[gi])
                bounces[gi] = bounces[gi] if bounces[gi] is not None else \
                    dram.tile([L * P, NT * P], FP8, name=f"bounce_g{gi}")
                nc.gpsimd.dma_start(
                    out=bounces[gi][slot * P:(slot + 1) * P, :],
                    in_=cT.rearrange("p t i -> p (t i)"))
                if slot == L - 1:
                    g = dram.tile([N_CORES * L * P, NT * P], FP8,
                                  name=f"gath{gi}", addr_space="Shared")
                    cc = nc.gpsimd.collective_compute(
                        "AllGather", mybir.AluOpType.bypass,
                        replica_groups=[list(range(N_CORES))],
                        ins=[bounces[gi].opt()], outs=[g.opt()],
                    )
                    cc_insts.append(cc.ins)
                    gath.append(g)

        # ---------- phase C: gathered reads + matmuls + epilogue ----------
        # Ordering guards: phase-C work on DVE/ACT must sit behind all
        # phase-B work on those queues, so late chunks are never stalled
        # behind epilogue ops waiting on a gather.
        last_dve = [last_dve_b[0]]
        last_act = [last_act_b[0]]
        for gi, grp in enumerate(GROUPS):
            L = len(grp)
            g_sb = gpool.tile([P, N_CORES * L, NT, P], FP8, name=f"gsb{gi}",
                              tag=f"gsb{gi}")
            # two rank-half reads: the h=0 matmuls need only ranks 0-3,
            # so they start ~2.5us before ranks 4-7 finish landing.
            gv = gath[gi].rearrange("(c p) f -> p c f", p=P).rearrange(
                "p c (t i) -> p c t i", t=NT)
            halfc = N_CORES * L // 2
            nc.sync.dma_start(out=g_sb[:, :halfc], in_=gv[:, :halfc])
            nc.sync.dma_start(out=g_sb[:, halfc:], in_=gv[:, halfc:])
            for q in range(NCHUNK):
                for slot in range(L):
                    ps = mpsum.tile([P, 2, N_CORES // 2 * P], F32,
                                    name=f"ps{gi}_{q}_{slot}", tag="ps")
                    for h in range(2):
                        for t in range(NT):
                            # rhs: ranks c in [4h,4h+4), chunk slot, d-tile t
                            cs = 4 * h * L + slot
                            rhs = g_sb[:, cs:cs + 3 * L + 1:L, t, :]
                            mm = nc.tensor.matmul(
                                ps[:, h, :],
                                uT[:, t, q * P:(q + 1) * P],
                                rhs,
                                start=(t == 0), stop=(t == NT - 1),
                            )
                            # keep every matmul behind all transposes in the
                            # PE queue so late-chunk transposes are never
                            # stalled by matmuls waiting on a gather.
                            if h == 0 and t == 0:
                                tile.add_dep_helper(
                                    mm.ins, last_transpose[0], sync=True,
                                    reason="PE: transposes before matmuls")
                    # epilogue: y = max(cos, eps); s += sum_k exp(w*y)
                    y = epool.tile([P, 2 * (N_CORES // 2) * P], BF16,
                                   name=f"y{gi}_{q}_{slot}", tag="y")
                    mx = nc.vector.tensor_scalar_max(
                        y, ps.rearrange("p a b -> p (a b)"), EPS)
                    e_scr = epool.tile([P, 2 * (N_CORES // 2) * P], BF16,
                                       name=f"escr{gi}_{q}_{slot}", tag="escr")
                    s_part = stats.tile([P, 1], F32, name=f"sp{gi}_{q}_{slot}",
                                        tag="sp")
                    ex = nc.scalar.activation(e_scr, y, AF.Exp, scale=w_val,
                                              accum_out=s_part)
                    nc.vector.tensor_add(s_acc[:, q:q + 1], s_acc[:, q:q + 1],
                                         s_part)

        # ---------- finals: ship s (exp-sums) and diag; host does the log
        nc.sync.dma_start(out=out[0].rearrange("(q p) -> p q", p=P), in_=s_acc)
        nc.sync.dma_start(out=out[1].rearrange("(q p) -> p q", p=P),
                          in_=diag_all)


_CACHE = {}


def kernel(dvecs, w, b):
    w_val = float(np.asarray(w))
    key = w_val
    if key not in _CACHE:
        _CACHE[key] = build_program(w_val)
    nc = _CACHE[key]
    dvecs = np.ascontiguousarray(np.asarray(dvecs, dtype=np.float32))
    in_maps = [
        {"dvecs": dvecs[c * LOCAL:(c + 1) * LOCAL]} for c in range(N_CORES)
    ]
    res = run_bass_kernel_spmd(nc, in_maps, core_ids=list(range(N_CORES)))
    total = 0.0
    for c in range(N_CORES):
        o = np.asarray(res.results[c]["out"], dtype=np.float64)
        s, diag = o[0], o[1]
        rows = np.log(s) - w_val * np.maximum(diag, EPS)
        total += float(rows.sum())
    return np.float32(total / N)
